# revision 3
# baseline (speedup 1.0000x reference)
"""nn_CosineSimilarity GNN edge kernel for 8x TRN2 NeuronCores.

Strategy (sharding_hint: shard edges across cores, replicate node table):
  - Host: group edges into 16 buckets by (src>>15, dst>>15) so int16 gather
    indices reach their table chunk; sort each bucket by src for HBM
    locality; deal each bucket evenly to the 8 cores (identical per-core
    bucket sizes -> one SPMD program); pad per-core buckets to multiples
    of 128.
  - Device (per core): dma_gather 512B rows of h for src and dst of each
    edge tile; cos = dot(hs,hd) * rsqrt(sum(hs^2)*sum(hd^2)) computed with
    DVE (mult + segmented reduces), ACT (squares, sqrt), all overlapped
    with the gathers by the Tile framework.
  - Host: inverse-permute per-core padded outputs back to edge order.
"""

import sys

sys.path.insert(0, '/opt/trn_rl_repo')

import numpy as np

import concourse.bacc as bacc
import concourse.bass as bass
import concourse.mybir as mybir
from concourse.tile import TileContext
from concourse import bass_utils, library_config

N, D, E = 100000, 128, 640000
NCORES = 8
P = 128
CH = 32768          # table chunk rows addressable by int16 gather indices
NCHUNK = (N + CH - 1) // CH
G = 2048            # max edges per dma_gather
SQ_DT = mybir.dt.float32   # dtype of squared tiles (fp16 halves DVE reduce time)


def _plan(src, dst):
    """Bucket/sort/shard edges. Returns per-core index arrays, tile list and
    the padded-position -> global-edge-id map."""
    src = np.asarray(src).astype(np.int64).ravel()
    dst = np.asarray(dst).astype(np.int64).ravel()
    e = src.shape[0]
    a = src >> 15
    b = dst >> 15
    key = a * NCHUNK + b
    order = np.lexsort((src, key))          # bucket-major, src-sorted inside
    key_sorted = key[order]
    bucket_starts = np.searchsorted(key_sorted, np.arange(NCHUNK * NCHUNK))
    bucket_ends = np.searchsorted(key_sorted, np.arange(NCHUNK * NCHUNK), side='right')

    # per-core padded layout (identical across cores)
    tiles = []          # (chunk_a, chunk_b, col16_off, gsize)
    btot = 0            # padded edges per core
    bucket_core_meta = []  # (g, start, end, B_g)
    for g in range(NCHUNK * NCHUNK):
        s0, s1 = int(bucket_starts[g]), int(bucket_ends[g])
        cnt = s1 - s0
        if cnt == 0:
            continue
        percore = -(-cnt // NCORES)             # ceil
        B_g = -(-percore // P) * P              # pad to multiple of 128
        bucket_core_meta.append((g, s0, s1, B_g))
        off = 0
        while off < B_g:
            gsz = min(G, B_g - off)
            tiles.append((g // NCHUNK, g % NCHUNK, btot + off, gsz))
            off += gsz
        btot += B_g

    sidx = np.zeros((NCORES, btot), np.int16)
    didx = np.zeros((NCORES, btot), np.int16)
    gid = np.full((NCORES, btot), -1, np.int64)

    pos = 0
    for g, s0, s1, B_g in bucket_core_meta:
        cnt = s1 - s0
        idxs = order[s0:s1]
        # contiguous near-equal slices keep the src sort per core
        splits = np.linspace(0, cnt, NCORES + 1).astype(np.int64)
        ca, cb = g // NCHUNK, g % NCHUNK
        for c in range(NCORES):
            sl = idxs[splits[c]:splits[c + 1]]
            n = sl.shape[0]
            sidx[c, pos:pos + n] = (src[sl] - ca * CH).astype(np.int16)
            didx[c, pos:pos + n] = (dst[sl] - cb * CH).astype(np.int16)
            gid[c, pos:pos + n] = sl
        pos += B_g
    assert pos == btot

    def wrap16(arr):            # [NCORES, btot] -> [NCORES, 128, btot//16]
        w = arr.reshape(NCORES, btot // 16, 16).transpose(0, 2, 1)  # [C,16,btot/16]
        return np.tile(w, (1, 8, 1)).astype(np.int16)

    return wrap16(sidx), wrap16(didx), gid, tiles, btot


def _build(tiles, btot, repeat=1, loop_repeat=1):
    """Build the SPMD Bass program (one NEFF, all cores identical).

    repeat statically unrolls the tile pass; loop_repeat wraps it in an
    on-device For_i (used by test.py to measure steady-state HW time)."""
    nc = bacc.Bacc("TRN2", target_bir_lowering=False, debug=False,
                   num_devices=NCORES)
    h = nc.dram_tensor("h", [N, D], mybir.dt.float32, kind="ExternalInput")
    sidx = nc.dram_tensor("sidx", [P, btot // 16], mybir.dt.int16, kind="ExternalInput")
    didx = nc.dram_tensor("didx", [P, btot // 16], mybir.dt.int16, kind="ExternalInput")
    out = nc.dram_tensor("out", [P, btot // P], mybir.dt.float32, kind="ExternalOutput")

    chunk_ap = [h[c * CH: min((c + 1) * CH, N), :] for c in range(NCHUNK)]

    with TileContext(nc) as tc:
        with (
            tc.tile_pool(name="idx", bufs=1) as idxp,
            tc.tile_pool(name="gat", bufs=3) as gp,
            tc.tile_pool(name="sq", bufs=3) as sqp,
            tc.tile_pool(name="small", bufs=4) as smp,
            tc.tile_pool(name="cosb", bufs=1) as cosp,
        ):
            nc.gpsimd.load_library(library_config.mlp)
            si = idxp.tile([P, btot // 16], mybir.dt.int16)
            di = idxp.tile([P, btot // 16], mybir.dt.int16)
            nc.sync.dma_start(out=si[:], in_=sidx[:])
            nc.sync.dma_start(out=di[:], in_=didx[:])
            cosbuf = cosp.tile([P, btot // P], mybir.dt.float32)

            from contextlib import nullcontext
            loop_ctx = (tc.For_i(0, loop_repeat, 1) if loop_repeat > 1
                        else nullcontext())
            with loop_ctx:
              for _ in range(repeat):
                for (ca, cb, off, gsz) in tiles:
                    m = gsz // P
                    io, c0 = off // 16, off // P
                    gs = gp.tile([P, m, D], mybir.dt.float32, tag="gs")
                    gd = gp.tile([P, m, D], mybir.dt.float32, tag="gd")
                    nc.gpsimd.dma_gather(gs[:], chunk_ap[ca], si[:, io:io + gsz // 16],
                                         gsz, gsz, D, single_packet=False)
                    nc.gpsimd.dma_gather(gd[:], chunk_ap[cb], di[:, io:io + gsz // 16],
                                         gsz, gsz, D, single_packet=False)
                    prod = sqp.tile([P, m, D], mybir.dt.float32, tag="prod")
                    s2 = sqp.tile([P, m, D], SQ_DT, tag="s2")
                    d2 = sqp.tile([P, m, D], SQ_DT, tag="d2")
                    nc.vector.tensor_tensor(
                        out=prod[:].rearrange("p a b -> p (a b)"),
                        in0=gs[:].rearrange("p a b -> p (a b)"),
                        in1=gd[:].rearrange("p a b -> p (a b)"),
                        op=mybir.AluOpType.mult)
                    nc.scalar.square(s2[:].rearrange("p a b -> p (a b)"),
                                     gs[:].rearrange("p a b -> p (a b)"))
                    nc.scalar.square(d2[:].rearrange("p a b -> p (a b)"),
                                     gd[:].rearrange("p a b -> p (a b)"))
                    dot = smp.tile([P, m], mybir.dt.float32, tag="dot")
                    ss = smp.tile([P, m], mybir.dt.float32, tag="ss")
                    sd = smp.tile([P, m], mybir.dt.float32, tag="sd")
                    nc.vector.tensor_reduce(dot[:], prod[:], axis=mybir.AxisListType.X,
                                            op=mybir.AluOpType.add)
                    nc.vector.tensor_reduce(ss[:], s2[:], axis=mybir.AxisListType.X,
                                            op=mybir.AluOpType.add)
                    nc.vector.tensor_reduce(sd[:], d2[:], axis=mybir.AxisListType.X,
                                            op=mybir.AluOpType.add)
                    ssd = smp.tile([P, m], mybir.dt.float32, tag="ssd")
                    nc.vector.tensor_tensor(out=ssd[:], in0=ss[:], in1=sd[:],
                                            op=mybir.AluOpType.mult)
                    den = smp.tile([P, m], mybir.dt.float32, tag="den")
                    nc.scalar.sqrt(den[:], ssd[:])
                    inv = smp.tile([P, m], mybir.dt.float32, tag="inv")
                    nc.vector.reciprocal(inv[:], den[:])
                    nc.vector.tensor_tensor(out=cosbuf[:, c0:c0 + m], in0=dot[:],
                                            in1=inv[:], op=mybir.AluOpType.mult)
            nc.sync.dma_start(out=out[:], in_=cosbuf[:])
    nc.compile()
    return nc


def kernel(h, src, dst):
    h = np.ascontiguousarray(np.asarray(h), dtype=np.float32)
    sidx_w, didx_w, gid, tiles, btot = _plan(src, dst)
    nc = _build(tiles, btot)
    in_maps = [
        {"h": h, "sidx": np.ascontiguousarray(sidx_w[c]),
         "didx": np.ascontiguousarray(didx_w[c])}
        for c in range(NCORES)
    ]
    res = bass_utils.run_bass_kernel_spmd(nc, in_maps, core_ids=list(range(NCORES)))
    full = np.zeros(E, np.float32)
    for c in range(NCORES):
        padded = res.results[c]["out"].T.ravel()     # padded-position order
        g = gid[c]
        valid = g >= 0
        full[g[valid]] = padded[valid]
    return full.reshape(E, 1)
